# revision 6
# baseline (speedup 1.0000x reference)
"""BiMatchLoss kernel for Trainium2 (8 NeuronCores, SPMD data-parallel over batch).

Math (validated vs reference):
  BCE(p,t) = -log1mp(p) - t*(logp(p) - log1mp(p))
  Summed over a bijective matching perm, the -log1mp part is perm-independent.
  Per batch b the device computes (one pass over the data):
    cost[t,o]  = -sum_{s,ci} tgt[s,t,ci] * out[s,o,ci]            (argmin input)
    G[t,o]     =  sum_{s,ci} tgt[s,t,ci] * mD[s,o,ci]
    Amask      =  sum_{s,o,ci} m[s] * (-log1mp[s,o,ci])
  where mD = m*(logp - log1mp). Host pre-masks the Ln inputs so the device
  computes m*logp = Ln(m*p + 1-m) and m*log1mp = Ln(m*(1-p) + 1-m) directly
  (the (1-p) form keeps fp8 inputs accurate where p -> 1).
  final = sum_b 0.5*(Amask_b - sum_t G[t, perm_b[t]]) / sum(m)

Device per batch: 2 Ln activations over [128,1536] fp8 inputs (ACT; the
log1mp one accumulates Amask row-sums), a subtract writing fp8 mD into the
comb rhs slots (DVE for batches 0/3, gpsimd for 1/2 to unload DVE), 8 fp8
DoubleRow matmuls (K=256 = two s-tiles per matmul, PSUM-accumulated over 4
double-tiles; the t4,t5 weight chunk is zero-padded to M=128 so all PSUM rows
are written), then one fused block-diag mask multiply + grouped fp16 reduce
-> [128,24] partials per batch. All input DMAs ride the sync queue as
contiguous per-partition lines in exact consumption order so the ACT chain
never stalls. Batch 3's log1mp/sub/matmuls are split 6+2 tiles to shorten the
serial tail. Host does the 720-permutation argmin and final scalar assembly.
"""

import os
from itertools import permutations

import numpy as np
import ml_dtypes

import concourse.bacc as bacc
import concourse.mybir as mybir
from concourse.tile import TileContext
from concourse.bass_utils import run_bass_kernel_spmd

B, S, E, C = 32, 1024, 6, 16
F = E * C * 2          # 192 flattened (e, c, i)
CI = C * 2             # 32
NCORE = 8
NB = B // NCORE        # 4 batches per core
NT = S // 128          # 8 s-tiles per batch
ND = NT // 2           # 4 double-tiles (K=256) per batch

f32 = mybir.dt.float32
f16 = mybir.dt.float16
bf16 = mybir.dt.bfloat16
fp8 = mybir.dt.float8e4
AF = mybir.ActivationFunctionType
ALU = mybir.AluOpType
AX = mybir.AxisListType
DR = mybir.MatmulPerfMode.DoubleRow

_PROG = None           # cached compiled Bass program
LAST = None            # last BassKernelResults (for test.py timing)


def _build_program():
    nc = bacc.Bacc("TRN2", target_bir_lowering=False, debug=False,
                   num_devices=1)

    lnin_d = nc.dram_tensor("lnin", [128, NB * 3072], fp8,
                            kind="ExternalInput").ap()
    xt_d = nc.dram_tensor("xt", [128, NB * 2048], fp8,
                          kind="ExternalInput").ap()
    xoc_d = nc.dram_tensor("xoc", [128, NB * 3072], fp8,
                           kind="ExternalInput").ap()
    dmask_d = nc.dram_tensor("dmask", [128, 768], bf16,
                             kind="ExternalInput").ap()
    red_d = nc.dram_tensor("red", [128, NB * 24], f16,
                           kind="ExternalOutput").ap()
    am_d = nc.dram_tensor("am", [128, 5], f32, kind="ExternalOutput").ap()

    with TileContext(nc) as tc:
        with (
            tc.tile_pool(name="sb", bufs=1) as sbp,
            tc.tile_pool(name="ps", bufs=1, space="PSUM") as psp,
        ):
            dmask_sb = sbp.tile([128, 768], bf16, tag="dmask")
            red_sb = sbp.tile([128, NB * 24], f16, tag="red")
            am_sb = sbp.tile([128, 5], f32, tag="am")

            lnin_sb, xt_sb, comb_sb, logs_sb, tmp_sb, ps_sb = (
                [], [], [], [], [], [])
            for b in range(NB):
                lnin_sb.append(sbp.tile([128, 3072], fp8, tag=f"lnin{b}",
                                        name=f"lnin{b}"))
                xt_sb.append(sbp.tile([128, 2048], fp8, tag=f"xt{b}",
                                      name=f"xt{b}"))
                comb_sb.append(sbp.tile([128, 3072], fp8, tag=f"comb{b}",
                                        name=f"comb{b}"))
                logs_sb.append(sbp.tile([128, 3072], bf16, tag=f"logs{b}",
                                        name=f"logs{b}"))
                tmp_sb.append(sbp.tile([128, 768], bf16, tag=f"tmp{b}",
                                       name=f"tmp{b}"))
                ps_sb.append(psp.tile([128, 1024], f32, tag=f"ps{b}",
                                      name=f"ps{b}"))

            # ---- phase A: all input DMAs on the sync queue, in exact
            # consumption order (one queue = strict FIFO = no cross-queue
            # interleave on the shared DMA-engine pipe).
            def dma_ln(b, half):
                lo = b * 3072 + half * 1536
                nc.sync.dma_start(lnin_sb[b][:, half * 1536:(half + 1) * 1536],
                                  lnin_d[:, lo:lo + 1536])

            def dma_comb(b):
                nc.sync.dma_start(comb_sb[b][:],
                                  xoc_d[:, b * 3072:(b + 1) * 3072])

            def dma_xt(b):
                nc.sync.dma_start(xt_sb[b][:],
                                  xt_d[:, b * 2048:(b + 1) * 2048])

            dma_ln(0, 0)
            dma_ln(0, 1)
            dma_ln(1, 0)
            dma_ln(1, 1)
            dma_comb(0)
            dma_xt(0)
            dma_ln(2, 0)
            dma_ln(2, 1)
            dma_comb(1)
            dma_xt(1)
            nc.sync.dma_start(dmask_sb[:], dmask_d)
            dma_ln(3, 0)
            dma_ln(3, 1)
            dma_comb(2)
            dma_xt(2)
            dma_comb(3)
            dma_xt(3)

            comb_vs = [comb_sb[b][:].rearrange("p (k q) -> p k q", q=384)
                       for b in range(NB)]

            # ---- phase B: per-batch compute
            def mms(b, dlo, dhi):
                xt_v = xt_sb[b][:].rearrange("p (k f) -> p k f", f=256)
                ps = ps_sb[b]
                for d in range(dlo, dhi):
                    st = dict(start=(d == 0), stop=(d == ND - 1))
                    rhs = comb_vs[b][:, 2 * d:2 * d + 2, :]
                    nc.tensor.matmul(ps[:, 0:384],
                                     xt_v[:, 2 * d:2 * d + 2, 0:128], rhs,
                                     perf_mode=DR, **st)
                    nc.tensor.matmul(ps[:, 512:896],
                                     xt_v[:, 2 * d:2 * d + 2, 128:256], rhs,
                                     perf_mode=DR, **st)

            def post(b):
                ps_v = ps_sb[b][:].rearrange(
                    "p (h q) -> p h q", q=512)[:, :, 0:384]
                nc.vector.tensor_tensor(tmp_sb[b][:], ps_v, dmask_sb[:],
                                        ALU.mult)
                with nc.allow_low_precision("24 partials of ~32-term block "
                                            "sums; fp16 keeps 2x DVE rate"):
                    nc.vector.tensor_reduce(
                        red_sb[:, b * 24:(b + 1) * 24],
                        tmp_sb[b][:].rearrange("p (g j) -> p g j", j=CI),
                        AX.X, ALU.add)

            def sub(b, tlo, thi, eng):
                logs = logs_sb[b]
                eng.tensor_sub(comb_vs[b][:, tlo:thi, F:384],
                               logs[:, tlo * F:thi * F],
                               logs[:, 1536 + tlo * F:1536 + thi * F])

            for b in range(NB):
                logs = logs_sb[b]
                lnin = lnin_sb[b]
                nc.scalar.activation(logs[:, 0:1536], lnin[:, 0:1536], AF.Ln)
                if b < NB - 1:
                    nc.scalar.activation(
                        logs[:, 1536:3072], lnin[:, 1536:3072], AF.Ln,
                        accum_out=am_sb[:, b:b + 1])
                    sub(b, 0, NT, nc.vector if b == 0 else nc.gpsimd)
                    mms(b, 0, ND)
                else:
                    # split the last batch 6+2 tiles so the serial tail after
                    # the ACT chain ends is one small sub + 2 matmuls + post
                    nc.scalar.activation(
                        logs[:, 1536:2688], lnin[:, 1536:2688], AF.Ln,
                        accum_out=am_sb[:, 3:4])
                    sub(b, 0, 6, nc.vector)
                    mms(b, 0, 3)
                    nc.scalar.activation(
                        logs[:, 2688:3072], lnin[:, 2688:3072], AF.Ln,
                        accum_out=am_sb[:, 4:5])
                    sub(b, 6, NT, nc.vector)
                    mms(b, 3, ND)
                if b > 0:
                    post(b - 1)
            nc.sync.dma_start(am_d, am_sb[:])
            post(NB - 1)

            # ---- phase C: final output DMA
            nc.sync.dma_start(red_d, red_sb[:])

    nc.compile()
    return nc


def _get_program():
    global _PROG
    if _PROG is None:
        _PROG = _build_program()
    return _PROG


def kernel(outputs, targets, attention_mask):
    global LAST
    out_np = np.asarray(outputs, dtype=np.float32).reshape(B, S, F)
    tgt_np = np.asarray(targets, dtype=np.float32).reshape(B, S, F)
    m_np = np.asarray(attention_mask)

    mf = m_np.astype(np.float32)[:, :, None]
    f8 = ml_dtypes.float8_e4m3fn
    # masked Ln inputs; binaries and masked copies are cheap host prep.
    # lnin = [xoo_b | xzo_b] per batch, in exact ACT consumption order.
    xoo_all = (out_np * mf + (1.0 - mf)).astype(f8)
    xzo_all = ((1.0 - out_np) * mf + (1.0 - mf)).astype(f8)
    lnin_all = np.concatenate(
        [xoo_all.reshape(B, 1, NT, 128, F),
         xzo_all.reshape(B, 1, NT, 128, F)], axis=1)  # [B, 2, NT, 128, F]
    # xt tiles zero-padded to 256 cols: [hi f0:128 | lo f128:192 | 64 zeros]
    xt_all = np.zeros((B, NT, 128, 256), dtype=f8)
    xt_all[:, :, :, 0:F] = tgt_np.astype(f8).reshape(B, NT, 128, F)
    # comb image: xo tiles in cols 0:192 of each 384 block, zeros in mD slots
    xoc_all = np.zeros((B, NT, 128, 384), dtype=f8)
    xoc_all[:, :, :, 0:F] = out_np.astype(f8).reshape(B, NT, 128, F)

    # dmask[p, q] = 1 where p%32 == q%32 (block-diagonal selector)
    p_idx = np.arange(128)[:, None] % CI
    q_idx = np.arange(768)[None, :] % CI
    dmask = (p_idx == q_idx).astype(ml_dtypes.bfloat16)

    in_maps = []
    for c in range(NCORE):
        bs = slice(c * NB, (c + 1) * NB)
        in_maps.append({
            "lnin": np.ascontiguousarray(
                lnin_all[bs].transpose(3, 0, 1, 2, 4).reshape(128, NB * 3072)),
            "xt": np.ascontiguousarray(
                xt_all[bs].transpose(2, 0, 1, 3).reshape(128, NB * 2048)),
            "xoc": np.ascontiguousarray(
                xoc_all[bs].transpose(2, 0, 1, 3).reshape(128, NB * 3072)),
            "dmask": dmask,
        })

    nc = _get_program()
    res = run_bass_kernel_spmd(nc, in_maps, list(range(NCORE)))
    LAST = res

    P = np.array(list(permutations(range(E))), dtype=np.int32)
    t_idx = np.arange(E)[None, :]
    ar = np.arange(E)
    num = 0.0
    for c in range(NCORE):
        red = res.results[c]["red"].astype(np.float32)  # [128, NB*24]
        am = res.results[c]["am"]                       # [128, 5] f32
        for b in range(NB):
            c0 = b * 24
            hi = red[:, c0:c0 + 12]          # rows (t0..3 x ci) x (o | o)
            lo = red[0:64, c0 + 12:c0 + 24]  # rows (t4,5 x ci)
            cost = -np.concatenate(
                [hi[:, 0:6].reshape(4, CI, 6).sum(1, dtype=np.float32),
                 lo[:, 0:6].reshape(2, CI, 6).sum(1, dtype=np.float32)],
                axis=0)
            G = np.concatenate(
                [hi[:, 6:12].reshape(4, CI, 6).sum(1, dtype=np.float32),
                 lo[:, 6:12].reshape(2, CI, 6).sum(1, dtype=np.float32)],
                axis=0)

            totals = cost[t_idx, P].sum(-1, dtype=np.float32)
            perm = P[int(np.argmin(totals))]
            num += -0.5 * float(G[ar, perm].sum(dtype=np.float64))
        num += 0.5 * -am.sum(dtype=np.float64)

    den = float(m_np.sum())
    return np.float32(num / den)


# revision 7
# speedup vs baseline: 1.0715x; 1.0715x over previous
"""BiMatchLoss kernel for Trainium2 (8 NeuronCores, SPMD data-parallel over batch).

Math (validated vs reference):
  BCE(p,t) = -log1mp(p) - t*(logp(p) - log1mp(p))
  Summed over a bijective matching perm, the -log1mp part is perm-independent.
  Per batch b the device computes (one pass over the data):
    cost[t,o]  = -sum_{s,ci} tgt[s,t,ci] * out[s,o,ci]            (argmin input)
    G[t,o]     =  sum_{s,ci} tgt[s,t,ci] * mD[s,o,ci]
    Amask      =  sum_{s,o,ci} m[s] * (-log1mp[s,o,ci])
  where mD = m*(logp - log1mp). Host pre-masks the Ln inputs so the device
  computes m*logp = Ln(m*p + 1-m) and m*log1mp = Ln(m*(1-p) + 1-m) directly
  (the (1-p) form keeps fp8 inputs accurate where p -> 1).
  final = sum_b 0.5*(Amask_b - sum_t G[t, perm_b[t]]) / sum(m)

Device per batch: 2 Ln activations over [128,1536] fp8 inputs (ACT; the
log1mp one accumulates Amask row-sums), a subtract writing fp8 mD into the
comb rhs slots (DVE for batches 0/3, gpsimd for 1/2 to unload DVE), 8 fp8
DoubleRow matmuls (K=256 = two s-tiles per matmul, PSUM-accumulated over 4
double-tiles; the t4,t5 weight chunk is zero-padded to M=128 so all PSUM rows
are written), then one fused block-diag mask multiply + grouped fp16 reduce
-> [128,24] partials per batch. All input DMAs ride the sync queue as
contiguous per-partition lines in exact consumption order so the ACT chain
never stalls. Batch 3's log1mp/sub/matmuls are split 6+2 tiles to shorten the
serial tail. Host does the 720-permutation argmin and final scalar assembly.
"""

import os
from itertools import permutations

import numpy as np
import ml_dtypes

import concourse.bacc as bacc
import concourse.mybir as mybir
from concourse.tile import TileContext
from concourse.bass_utils import run_bass_kernel_spmd

B, S, E, C = 32, 1024, 6, 16
F = E * C * 2          # 192 flattened (e, c, i)
CI = C * 2             # 32
NCORE = 8
NB = B // NCORE        # 4 batches per core
NT = S // 128          # 8 s-tiles per batch
ND = NT // 2           # 4 double-tiles (K=256) per batch

f32 = mybir.dt.float32
f16 = mybir.dt.float16
bf16 = mybir.dt.bfloat16
fp8 = mybir.dt.float8e4
AF = mybir.ActivationFunctionType
ALU = mybir.AluOpType
AX = mybir.AxisListType
DR = mybir.MatmulPerfMode.DoubleRow

_PROG = None           # cached compiled Bass program
LAST = None            # last BassKernelResults (for test.py timing)


def _build_program():
    nc = bacc.Bacc("TRN2", target_bir_lowering=False, debug=False,
                   num_devices=1)

    lnin_d = nc.dram_tensor("lnin", [128, NB * 3072], fp8,
                            kind="ExternalInput").ap()
    xt_d = nc.dram_tensor("xt", [128, NB * 2048], fp8,
                          kind="ExternalInput").ap()
    xoc_d = nc.dram_tensor("xoc", [128, NB * 3072], fp8,
                           kind="ExternalInput").ap()
    dmask_d = nc.dram_tensor("dmask", [128, 768], bf16,
                             kind="ExternalInput").ap()
    red_d = nc.dram_tensor("red", [128, NB * 24], f16,
                           kind="ExternalOutput").ap()
    am_d = nc.dram_tensor("am", [128, 5], f32, kind="ExternalOutput").ap()

    with TileContext(nc) as tc:
        with (
            tc.tile_pool(name="sb", bufs=1) as sbp,
            tc.tile_pool(name="ps", bufs=1, space="PSUM") as psp,
        ):
            dmask_sb = sbp.tile([128, 768], bf16, tag="dmask")
            red_sb = sbp.tile([128, NB * 24], f16, tag="red")
            am_sb = sbp.tile([128, 5], f32, tag="am")

            lnin_sb, xt_sb, comb_sb, logs_sb, tmp_sb, ps_sb = (
                [], [], [], [], [], [])
            for b in range(NB):
                lnin_sb.append(sbp.tile([128, 3072], fp8, tag=f"lnin{b}",
                                        name=f"lnin{b}"))
                xt_sb.append(sbp.tile([128, 2048], fp8, tag=f"xt{b}",
                                      name=f"xt{b}"))
                comb_sb.append(sbp.tile([128, 3072], fp8, tag=f"comb{b}",
                                        name=f"comb{b}"))
                logs_sb.append(sbp.tile([128, 3072], bf16, tag=f"logs{b}",
                                        name=f"logs{b}"))
                tmp_sb.append(sbp.tile([128, 768], bf16, tag=f"tmp{b}",
                                       name=f"tmp{b}"))
                ps_sb.append(psp.tile([128, 1024], f32, tag=f"ps{b}",
                                      name=f"ps{b}"))

            # ---- phase A: all input DMAs on the sync queue, in exact
            # consumption order (one queue = strict FIFO = no cross-queue
            # interleave on the shared DMA-engine pipe).
            def dma_ln(b, half):
                lo = b * 3072 + half * 1536
                nc.sync.dma_start(lnin_sb[b][:, half * 1536:(half + 1) * 1536],
                                  lnin_d[:, lo:lo + 1536])

            def dma_comb(b):
                nc.sync.dma_start(comb_sb[b][:],
                                  xoc_d[:, b * 3072:(b + 1) * 3072])

            def dma_xt(b):
                nc.sync.dma_start(xt_sb[b][:],
                                  xt_d[:, b * 2048:(b + 1) * 2048])

            dma_ln(0, 0)
            dma_ln(0, 1)
            dma_ln(1, 0)
            dma_ln(1, 1)
            dma_comb(0)
            dma_xt(0)
            dma_ln(2, 0)
            dma_ln(2, 1)
            dma_comb(1)
            dma_xt(1)
            nc.sync.dma_start(dmask_sb[:], dmask_d)
            dma_ln(3, 0)
            dma_ln(3, 1)
            dma_comb(2)
            dma_xt(2)
            dma_comb(3)
            dma_xt(3)

            comb_vs = [comb_sb[b][:].rearrange("p (k q) -> p k q", q=384)
                       for b in range(NB)]

            # ---- phase B: per-batch compute
            def mms(b, dlo, dhi):
                xt_v = xt_sb[b][:].rearrange("p (k f) -> p k f", f=256)
                ps = ps_sb[b]
                for d in range(dlo, dhi):
                    st = dict(start=(d == 0), stop=(d == ND - 1))
                    rhs = comb_vs[b][:, 2 * d:2 * d + 2, :]
                    nc.tensor.matmul(ps[:, 0:384],
                                     xt_v[:, 2 * d:2 * d + 2, 0:128], rhs,
                                     perf_mode=DR, **st)
                    nc.tensor.matmul(ps[:, 512:896],
                                     xt_v[:, 2 * d:2 * d + 2, 128:256], rhs,
                                     perf_mode=DR, **st)

            def post(b):
                ps_v = ps_sb[b][:].rearrange(
                    "p (h q) -> p h q", q=512)[:, :, 0:384]
                nc.vector.tensor_tensor(tmp_sb[b][:], ps_v, dmask_sb[:],
                                        ALU.mult)
                with nc.allow_low_precision("24 partials of ~32-term block "
                                            "sums; fp16 keeps 2x DVE rate"):
                    nc.vector.tensor_reduce(
                        red_sb[:, b * 24:(b + 1) * 24],
                        tmp_sb[b][:].rearrange("p (g j) -> p g j", j=CI),
                        AX.X, ALU.add)

            def sub(b, tlo, thi, eng):
                logs = logs_sb[b]
                eng.tensor_sub(comb_vs[b][:, tlo:thi, F:384],
                               logs[:, tlo * F:thi * F],
                               logs[:, 1536 + tlo * F:1536 + thi * F])

            for b in range(NB):
                logs = logs_sb[b]
                lnin = lnin_sb[b]
                nc.scalar.activation(logs[:, 0:1536], lnin[:, 0:1536], AF.Ln)
                if b < NB - 1:
                    nc.scalar.activation(
                        logs[:, 1536:3072], lnin[:, 1536:3072], AF.Ln,
                        accum_out=am_sb[:, b:b + 1])
                    # split: front half on DVE, back half on gpsimd (slower
                    # per element but a parallel engine; unloads DVE)
                    sub(b, 0, 4, nc.vector)
                    sub(b, 4, NT, nc.gpsimd)
                    mms(b, 0, ND)
                else:
                    # split the last batch 6+2 tiles so the serial tail after
                    # the ACT chain ends is one small sub + 2 matmuls + post
                    nc.scalar.activation(
                        logs[:, 1536:2688], lnin[:, 1536:2688], AF.Ln,
                        accum_out=am_sb[:, 3:4])
                    sub(b, 0, 4, nc.vector)
                    sub(b, 4, 6, nc.gpsimd)
                    mms(b, 0, 3)
                    nc.scalar.activation(
                        logs[:, 2688:3072], lnin[:, 2688:3072], AF.Ln,
                        accum_out=am_sb[:, 4:5])
                    sub(b, 6, NT, nc.vector)
                    mms(b, 3, ND)
                if b > 0:
                    post(b - 1)
            nc.sync.dma_start(am_d, am_sb[:])
            post(NB - 1)

            # ---- phase C: final output DMA
            nc.sync.dma_start(red_d, red_sb[:])

    nc.compile()
    return nc


def _get_program():
    global _PROG
    if _PROG is None:
        _PROG = _build_program()
    return _PROG


def kernel(outputs, targets, attention_mask):
    global LAST
    out_np = np.asarray(outputs, dtype=np.float32).reshape(B, S, F)
    tgt_np = np.asarray(targets, dtype=np.float32).reshape(B, S, F)
    m_np = np.asarray(attention_mask)

    mf = m_np.astype(np.float32)[:, :, None]
    f8 = ml_dtypes.float8_e4m3fn
    # masked Ln inputs; binaries and masked copies are cheap host prep.
    # lnin = [xoo_b | xzo_b] per batch, in exact ACT consumption order.
    xoo_all = (out_np * mf + (1.0 - mf)).astype(f8)
    xzo_all = ((1.0 - out_np) * mf + (1.0 - mf)).astype(f8)
    lnin_all = np.concatenate(
        [xoo_all.reshape(B, 1, NT, 128, F),
         xzo_all.reshape(B, 1, NT, 128, F)], axis=1)  # [B, 2, NT, 128, F]
    # xt tiles zero-padded to 256 cols: [hi f0:128 | lo f128:192 | 64 zeros]
    xt_all = np.zeros((B, NT, 128, 256), dtype=f8)
    xt_all[:, :, :, 0:F] = tgt_np.astype(f8).reshape(B, NT, 128, F)
    # comb image: xo tiles in cols 0:192 of each 384 block, zeros in mD slots
    xoc_all = np.zeros((B, NT, 128, 384), dtype=f8)
    xoc_all[:, :, :, 0:F] = out_np.astype(f8).reshape(B, NT, 128, F)

    # dmask[p, q] = 1 where p%32 == q%32 (block-diagonal selector)
    p_idx = np.arange(128)[:, None] % CI
    q_idx = np.arange(768)[None, :] % CI
    dmask = (p_idx == q_idx).astype(ml_dtypes.bfloat16)

    in_maps = []
    for c in range(NCORE):
        bs = slice(c * NB, (c + 1) * NB)
        in_maps.append({
            "lnin": np.ascontiguousarray(
                lnin_all[bs].transpose(3, 0, 1, 2, 4).reshape(128, NB * 3072)),
            "xt": np.ascontiguousarray(
                xt_all[bs].transpose(2, 0, 1, 3).reshape(128, NB * 2048)),
            "xoc": np.ascontiguousarray(
                xoc_all[bs].transpose(2, 0, 1, 3).reshape(128, NB * 3072)),
            "dmask": dmask,
        })

    nc = _get_program()
    res = run_bass_kernel_spmd(nc, in_maps, list(range(NCORE)))
    LAST = res

    P = np.array(list(permutations(range(E))), dtype=np.int32)
    t_idx = np.arange(E)[None, :]
    ar = np.arange(E)
    num = 0.0
    for c in range(NCORE):
        red = res.results[c]["red"].astype(np.float32)  # [128, NB*24]
        am = res.results[c]["am"]                       # [128, 5] f32
        for b in range(NB):
            c0 = b * 24
            hi = red[:, c0:c0 + 12]          # rows (t0..3 x ci) x (o | o)
            lo = red[0:64, c0 + 12:c0 + 24]  # rows (t4,5 x ci)
            cost = -np.concatenate(
                [hi[:, 0:6].reshape(4, CI, 6).sum(1, dtype=np.float32),
                 lo[:, 0:6].reshape(2, CI, 6).sum(1, dtype=np.float32)],
                axis=0)
            G = np.concatenate(
                [hi[:, 6:12].reshape(4, CI, 6).sum(1, dtype=np.float32),
                 lo[:, 6:12].reshape(2, CI, 6).sum(1, dtype=np.float32)],
                axis=0)

            totals = cost[t_idx, P].sum(-1, dtype=np.float32)
            perm = P[int(np.argmin(totals))]
            num += -0.5 * float(G[ar, perm].sum(dtype=np.float64))
        num += 0.5 * -am.sum(dtype=np.float64)

    den = float(m_np.sum())
    return np.float32(num / den)


# revision 8
# speedup vs baseline: 1.1689x; 1.0909x over previous
"""BiMatchLoss kernel for Trainium2 (8 NeuronCores, SPMD data-parallel over batch).

Math (validated vs reference):
  BCE(p,t) = -log1mp(p) - t*(logp(p) - log1mp(p))
  Summed over a bijective matching perm, the -log1mp part is perm-independent.
  Per batch b the device computes (one pass over the data):
    cost[t,o]  = -sum_{s,ci} tgt[s,t,ci] * out[s,o,ci]            (argmin input)
    G[t,o]     =  sum_{s,ci} tgt[s,t,ci] * mD[s,o,ci]
    Amask      =  sum_{s,o,ci} m[s] * (-log1mp[s,o,ci])
  where mD = m*(logp - log1mp). Host pre-masks the Ln inputs so the device
  computes m*logp = Ln(m*p + 1-m) and m*log1mp = Ln(m*(1-p) + 1-m) directly
  (the (1-p) form keeps fp8 inputs accurate where p -> 1).
  final = sum_b 0.5*(Amask_b - sum_t G[t, perm_b[t]]) / sum(m)

Device per batch: 2 Ln activations over [128,1536] fp8 inputs (ACT; the
log1mp one accumulates Amask row-sums), a subtract writing fp8 mD into the
comb rhs slots, 8 fp8 DoubleRow matmuls (K=256 = two s-tiles per matmul,
PSUM-accumulated over 4 double-tiles; the t4,t5 weight chunk is zero-padded to
M=128 so every PSUM row is written), then ONE PSUM->SBUF f16 copy. The
block-diagonal extraction needs no on-device reduce at all: the wanted
entry per partition is the single element PS[p, g*32 + p%32], so the host
just fancy-indexes the copied [128,768] panes and sums 32 rows. All input
DMAs ride the sync queue as contiguous per-partition lines in exact
consumption order so the ACT chain never stalls. Batch 3's log1mp/sub/
matmuls are split 6+2 tiles and its PSUM copy runs on the then-idle ACT
engine to shorten the serial tail. Host does the 720-permutation argmin and
final scalar assembly.
"""

import os
from itertools import permutations

import numpy as np
import ml_dtypes

import concourse.bacc as bacc
import concourse.mybir as mybir
from concourse.tile import TileContext
from concourse.bass_utils import run_bass_kernel_spmd

B, S, E, C = 32, 1024, 6, 16
F = E * C * 2          # 192 flattened (e, c, i)
CI = C * 2             # 32
NCORE = 8
NB = B // NCORE        # 4 batches per core
NT = S // 128          # 8 s-tiles per batch
ND = NT // 2           # 4 double-tiles (K=256) per batch

f32 = mybir.dt.float32
f16 = mybir.dt.float16
bf16 = mybir.dt.bfloat16
fp8 = mybir.dt.float8e4
AF = mybir.ActivationFunctionType
ALU = mybir.AluOpType
DR = mybir.MatmulPerfMode.DoubleRow

_PROG = None           # cached compiled Bass program
LAST = None            # last BassKernelResults (for test.py timing)


def _build_program():
    nc = bacc.Bacc("TRN2", target_bir_lowering=False, debug=False,
                   num_devices=1)

    lnin_d = nc.dram_tensor("lnin", [128, NB * 3072], fp8,
                            kind="ExternalInput").ap()
    xt_d = nc.dram_tensor("xt", [128, NB * 2048], fp8,
                          kind="ExternalInput").ap()
    xoc_d = nc.dram_tensor("xoc", [128, NB * 3072], fp8,
                           kind="ExternalInput").ap()
    cop_d = nc.dram_tensor("cop", [128, NB * 768], f16,
                           kind="ExternalOutput").ap()
    am_d = nc.dram_tensor("am", [128, 5], f32, kind="ExternalOutput").ap()

    with TileContext(nc) as tc:
        with (
            tc.tile_pool(name="sb", bufs=1) as sbp,
            tc.tile_pool(name="ps", bufs=1, space="PSUM") as psp,
        ):
            am_sb = sbp.tile([128, 5], f32, tag="am")

            lnin_sb, xt_sb, comb_sb, logs_sb, cop_sb, ps_sb = (
                [], [], [], [], [], [])
            for b in range(NB):
                lnin_sb.append(sbp.tile([128, 3072], fp8, tag=f"lnin{b}",
                                        name=f"lnin{b}"))
                xt_sb.append(sbp.tile([128, 2048], fp8, tag=f"xt{b}",
                                      name=f"xt{b}"))
                comb_sb.append(sbp.tile([128, 3072], fp8, tag=f"comb{b}",
                                        name=f"comb{b}"))
                logs_sb.append(sbp.tile([128, 3072], bf16, tag=f"logs{b}",
                                        name=f"logs{b}"))
                cop_sb.append(sbp.tile([128, 768], f16, tag=f"cop{b}",
                                       name=f"cop{b}"))
                ps_sb.append(psp.tile([128, 1024], f32, tag=f"ps{b}",
                                      name=f"ps{b}"))

            # ---- phase A: all input DMAs on the sync queue, in exact
            # consumption order (one queue = strict FIFO = no cross-queue
            # interleave on the shared DMA-engine pipe).
            def dma_ln(b, half):
                lo = b * 3072 + half * 1536
                nc.sync.dma_start(lnin_sb[b][:, half * 1536:(half + 1) * 1536],
                                  lnin_d[:, lo:lo + 1536])

            def dma_comb(b):
                nc.sync.dma_start(comb_sb[b][:],
                                  xoc_d[:, b * 3072:(b + 1) * 3072])

            def dma_xt(b):
                nc.sync.dma_start(xt_sb[b][:],
                                  xt_d[:, b * 2048:(b + 1) * 2048])

            dma_ln(0, 0)
            dma_ln(0, 1)
            dma_ln(1, 0)
            dma_ln(1, 1)
            dma_comb(0)
            dma_xt(0)
            dma_ln(2, 0)
            dma_ln(2, 1)
            dma_comb(1)
            dma_xt(1)
            dma_ln(3, 0)
            dma_ln(3, 1)
            dma_comb(2)
            dma_xt(2)
            dma_comb(3)
            dma_xt(3)

            comb_vs = [comb_sb[b][:].rearrange("p (k q) -> p k q", q=384)
                       for b in range(NB)]

            # ---- phase B: per-batch compute
            def mms(b, dlo, dhi):
                xt_v = xt_sb[b][:].rearrange("p (k f) -> p k f", f=256)
                ps = ps_sb[b]
                for d in range(dlo, dhi):
                    st = dict(start=(d == 0), stop=(d == ND - 1))
                    rhs = comb_vs[b][:, 2 * d:2 * d + 2, :]
                    nc.tensor.matmul(ps[:, 0:384],
                                     xt_v[:, 2 * d:2 * d + 2, 0:128], rhs,
                                     perf_mode=DR, **st)
                    nc.tensor.matmul(ps[:, 512:896],
                                     xt_v[:, 2 * d:2 * d + 2, 128:256], rhs,
                                     perf_mode=DR, **st)

            def ps_view(b):
                return ps_sb[b][:].rearrange(
                    "p (h q) -> p h q", q=512)[:, :, 0:384]

            def sub(b, tlo, thi, eng):
                logs = logs_sb[b]
                eng.tensor_sub(comb_vs[b][:, tlo:thi, F:384],
                               logs[:, tlo * F:thi * F],
                               logs[:, 1536 + tlo * F:1536 + thi * F])

            for b in range(NB):
                logs = logs_sb[b]
                lnin = lnin_sb[b]
                nc.scalar.activation(logs[:, 0:1536], lnin[:, 0:1536], AF.Ln)
                if b < NB - 1:
                    nc.scalar.activation(
                        logs[:, 1536:3072], lnin[:, 1536:3072], AF.Ln,
                        accum_out=am_sb[:, b:b + 1])
                    sub(b, 0, NT, nc.vector)
                    mms(b, 0, ND)
                else:
                    # split the last batch 6+2 tiles so the serial tail after
                    # the ACT chain ends is one small sub + 2 matmuls + copy
                    nc.scalar.activation(
                        logs[:, 1536:2688], lnin[:, 1536:2688], AF.Ln,
                        accum_out=am_sb[:, 3:4])
                    sub(b, 0, 6, nc.vector)
                    mms(b, 0, 3)
                    nc.scalar.activation(
                        logs[:, 2688:3072], lnin[:, 2688:3072], AF.Ln,
                        accum_out=am_sb[:, 4:5])
                    sub(b, 6, NT, nc.vector)
                    mms(b, 3, ND)
                if b > 0:
                    nc.vector.tensor_copy(cop_sb[b - 1][:], ps_view(b - 1))
            # batch 3's copy on the ACT engine, idle once its Ln chain ends
            nc.scalar.copy(cop_sb[NB - 1][:], ps_view(NB - 1))

            # ---- phase C: output DMAs
            nc.sync.dma_start(am_d, am_sb[:])
            for b in range(NB):
                nc.sync.dma_start(cop_d[:, b * 768:(b + 1) * 768],
                                  cop_sb[b][:])

    nc.compile()
    return nc


def _get_program():
    global _PROG
    if _PROG is None:
        _PROG = _build_program()
    return _PROG


def kernel(outputs, targets, attention_mask):
    global LAST
    out_np = np.asarray(outputs, dtype=np.float32).reshape(B, S, F)
    tgt_np = np.asarray(targets, dtype=np.float32).reshape(B, S, F)
    m_np = np.asarray(attention_mask)

    mf = m_np.astype(np.float32)[:, :, None]
    f8 = ml_dtypes.float8_e4m3fn
    # masked Ln inputs; binaries and masked copies are cheap host prep.
    # lnin = [xoo_b | xzo_b] per batch, in exact ACT consumption order.
    xoo_all = (out_np * mf + (1.0 - mf)).astype(f8)
    xzo_all = ((1.0 - out_np) * mf + (1.0 - mf)).astype(f8)
    lnin_all = np.concatenate(
        [xoo_all.reshape(B, 1, NT, 128, F),
         xzo_all.reshape(B, 1, NT, 128, F)], axis=1)  # [B, 2, NT, 128, F]
    # xt tiles zero-padded to 256 cols: [hi f0:128 | lo f128:192 | 64 zeros]
    xt_all = np.zeros((B, NT, 128, 256), dtype=f8)
    xt_all[:, :, :, 0:F] = tgt_np.astype(f8).reshape(B, NT, 128, F)
    # comb image: xo tiles in cols 0:192 of each 384 block, zeros in mD slots
    xoc_all = np.zeros((B, NT, 128, 384), dtype=f8)
    xoc_all[:, :, :, 0:F] = out_np.astype(f8).reshape(B, NT, 128, F)

    in_maps = []
    for c in range(NCORE):
        bs = slice(c * NB, (c + 1) * NB)
        in_maps.append({
            "lnin": np.ascontiguousarray(
                lnin_all[bs].transpose(3, 0, 1, 2, 4).reshape(128, NB * 3072)),
            "xt": np.ascontiguousarray(
                xt_all[bs].transpose(2, 0, 1, 3).reshape(128, NB * 2048)),
            "xoc": np.ascontiguousarray(
                xoc_all[bs].transpose(2, 0, 1, 3).reshape(128, NB * 3072)),
        })

    nc = _get_program()
    res = run_bass_kernel_spmd(nc, in_maps, list(range(NCORE)))
    LAST = res

    P = np.array(list(permutations(range(E))), dtype=np.int32)
    t_idx = np.arange(E)[None, :]
    ar = np.arange(E)
    p_arange = np.arange(128)
    diag = p_arange[:, None] % CI + np.arange(E)[None, :] * CI  # [128, 6]
    num = 0.0
    for c in range(NCORE):
        cop = res.results[c]["cop"].astype(np.float32)  # [128, NB*768]
        am = res.results[c]["am"]                       # [128, 5] f32
        for b in range(NB):
            pane = cop[:, b * 768:(b + 1) * 768]
            # pane cols: [cost-hi 0:192 | G-hi 192:384 | cost-lo | G-lo]
            # wanted entry per partition: col g*32 + p%32 of each block
            ch = pane[p_arange[:, None], diag]               # [128, 6]
            gh = pane[p_arange[:, None], 192 + diag]
            cl = pane[p_arange[:64, None], 384 + diag[:64]]  # [64, 6]
            gl = pane[p_arange[:64, None], 576 + diag[:64]]
            cost = -np.concatenate(
                [ch.reshape(4, CI, 6).sum(1, dtype=np.float32),
                 cl.reshape(2, CI, 6).sum(1, dtype=np.float32)], axis=0)
            G = np.concatenate(
                [gh.reshape(4, CI, 6).sum(1, dtype=np.float32),
                 gl.reshape(2, CI, 6).sum(1, dtype=np.float32)], axis=0)

            totals = cost[t_idx, P].sum(-1, dtype=np.float32)
            perm = P[int(np.argmin(totals))]
            num += -0.5 * float(G[ar, perm].sum(dtype=np.float64))
        num += 0.5 * -am.sum(dtype=np.float64)

    den = float(m_np.sum())
    return np.float32(num / den)


# revision 10
# speedup vs baseline: 1.3325x; 1.1399x over previous
"""BiMatchLoss kernel for Trainium2 (8 NeuronCores, SPMD data-parallel over batch).

Math (validated vs reference):
  BCE(p,t) = -log1mp(p) - t*(logp(p) - log1mp(p))
  Summed over a bijective matching perm, the -log1mp part is perm-independent.
  Per batch b the device computes (one pass over the data):
    cost[t,o]  = -sum_{s,ci} tgt[s,t,ci] * out[s,o,ci]            (argmin input)
    G[t,o]     =  sum_{s,ci} tgt[s,t,ci] * mD[s,o,ci]
    Amask      =  sum_{s,o,ci} m[s] * (-log1mp[s,o,ci])
  where mD = m*(logp - log1mp). Host pre-masks the Ln inputs so the device
  computes m*logp = Ln(m*p + 1-m) and m*log1mp = Ln(m*(1-p) + 1-m) directly
  (the (1-p) form keeps fp8 inputs accurate where p -> 1).
  final = sum_b 0.5*(Amask_b - sum_t G[t, perm_b[t]]) / sum(m)

Device per batch: 2 Ln activations over [128,1536] fp8 inputs (ACT; the
log1mp one accumulates Amask row-sums), a subtract writing fp8 mD into the
comb rhs slots, 8 fp8 DoubleRow matmuls (K=256 = two s-tiles per matmul,
PSUM-accumulated over 4 double-tiles; the t4,t5 weight chunk is zero-padded to
M=128 so every PSUM row is written), then ONE PSUM->SBUF f16 copy. The
block-diagonal extraction needs no on-device reduce at all: the wanted
entry per partition is the single element PS[p, g*32 + p%32], so the host
just fancy-indexes the copied [128,768] panes and sums 32 rows. All input
DMAs ride the sync queue as contiguous per-partition lines in exact
consumption order so the ACT chain never stalls. Batch 3's log1mp/sub/
matmuls are split 6+2 tiles and its PSUM copy runs on the then-idle ACT
engine to shorten the serial tail. Host does the 720-permutation argmin and
final scalar assembly.
"""

import os
from itertools import permutations

import numpy as np
import ml_dtypes

import concourse.bacc as bacc
import concourse.mybir as mybir
from concourse.tile import TileContext
from concourse.bass_utils import run_bass_kernel_spmd

B, S, E, C = 32, 1024, 6, 16
F = E * C * 2          # 192 flattened (e, c, i)
CI = C * 2             # 32
NCORE = 8
NB = B // NCORE        # 4 batches per core
NT = S // 128          # 8 s-tiles per batch
ND = NT // 2           # 4 double-tiles (K=256) per batch

f32 = mybir.dt.float32
f16 = mybir.dt.float16
bf16 = mybir.dt.bfloat16
fp8 = mybir.dt.float8e4
AF = mybir.ActivationFunctionType
ALU = mybir.AluOpType
DR = mybir.MatmulPerfMode.DoubleRow

_PROG = None           # cached compiled Bass program
LAST = None            # last BassKernelResults (for test.py timing)


def _build_program():
    nc = bacc.Bacc("TRN2", target_bir_lowering=False, debug=False,
                   num_devices=1)

    lnin_d = nc.dram_tensor("lnin", [128, NB * 3072], fp8,
                            kind="ExternalInput").ap()
    xt_d = nc.dram_tensor("xt", [128, NB * 2048], fp8,
                          kind="ExternalInput").ap()
    xoc_d = nc.dram_tensor("xoc", [128, NB * 3072], fp8,
                           kind="ExternalInput").ap()
    cop_d = nc.dram_tensor("cop", [128, NB * 768], f16,
                           kind="ExternalOutput").ap()
    am_d = nc.dram_tensor("am", [128, 5], f32, kind="ExternalOutput").ap()

    with TileContext(nc) as tc:
        with (
            tc.tile_pool(name="sb", bufs=1) as sbp,
            tc.tile_pool(name="ps", bufs=1, space="PSUM") as psp,
        ):
            am_sb = sbp.tile([128, 5], f32, tag="am")

            lnin_sb, xt_sb, comb_sb, logs_sb, cop_sb, ps_sb = (
                [], [], [], [], [], [])
            for b in range(NB):
                lnin_sb.append(sbp.tile([128, 3072], fp8, tag=f"lnin{b}",
                                        name=f"lnin{b}"))
                xt_sb.append(sbp.tile([128, 2048], fp8, tag=f"xt{b}",
                                      name=f"xt{b}"))
                comb_sb.append(sbp.tile([128, 3072], fp8, tag=f"comb{b}",
                                        name=f"comb{b}"))
                logs_sb.append(sbp.tile([128, 3072], bf16, tag=f"logs{b}",
                                        name=f"logs{b}"))
                cop_sb.append(sbp.tile([128, 768], f16, tag=f"cop{b}",
                                       name=f"cop{b}"))
                ps_sb.append(psp.tile([128, 1024], f32, tag=f"ps{b}",
                                      name=f"ps{b}"))

            # ---- phase A: all input DMAs on the sync queue, in exact
            # consumption order (one queue = strict FIFO = no cross-queue
            # interleave on the shared DMA-engine pipe).
            def dma_ln(b, half):
                lo = b * 3072 + half * 1536
                nc.sync.dma_start(lnin_sb[b][:, half * 1536:(half + 1) * 1536],
                                  lnin_d[:, lo:lo + 1536])

            def dma_comb(b):
                nc.sync.dma_start(comb_sb[b][:],
                                  xoc_d[:, b * 3072:(b + 1) * 3072])

            def dma_xt(b):
                nc.sync.dma_start(xt_sb[b][:],
                                  xt_d[:, b * 2048:(b + 1) * 2048])

            dma_ln(0, 0)
            dma_ln(0, 1)
            dma_ln(1, 0)
            dma_ln(1, 1)
            dma_comb(0)
            dma_xt(0)
            dma_ln(2, 0)
            dma_ln(2, 1)
            dma_comb(1)
            dma_xt(1)
            dma_ln(3, 0)
            dma_ln(3, 1)
            dma_comb(2)
            dma_xt(2)
            dma_comb(3)
            dma_xt(3)

            comb_vs = [comb_sb[b][:].rearrange("p (k q) -> p k q", q=384)
                       for b in range(NB)]

            # ---- phase B: per-batch compute
            def mms(b, dlo, dhi):
                xt_v = xt_sb[b][:].rearrange("p (k f) -> p k f", f=256)
                ps = ps_sb[b]
                for d in range(dlo, dhi):
                    st = dict(start=(d == 0), stop=(d == ND - 1))
                    rhs = comb_vs[b][:, 2 * d:2 * d + 2, :]
                    nc.tensor.matmul(ps[:, 0:384],
                                     xt_v[:, 2 * d:2 * d + 2, 0:128], rhs,
                                     perf_mode=DR, **st)
                    nc.tensor.matmul(ps[:, 512:896],
                                     xt_v[:, 2 * d:2 * d + 2, 128:256], rhs,
                                     perf_mode=DR, **st)

            def ps_view(b):
                return ps_sb[b][:].rearrange(
                    "p (h q) -> p h q", q=512)[:, :, 0:384]

            def sub(b, tlo, thi, eng):
                logs = logs_sb[b]
                eng.tensor_sub(comb_vs[b][:, tlo:thi, F:384],
                               logs[:, tlo * F:thi * F],
                               logs[:, 1536 + tlo * F:1536 + thi * F])

            for b in range(NB):
                logs = logs_sb[b]
                lnin = lnin_sb[b]
                nc.scalar.activation(logs[:, 0:1536], lnin[:, 0:1536], AF.Ln)
                if b < NB - 1:
                    nc.scalar.activation(
                        logs[:, 1536:3072], lnin[:, 1536:3072], AF.Ln,
                        accum_out=am_sb[:, b:b + 1])
                    sub(b, 0, NT, nc.vector)
                    # previous batch's PSUM copy issues AFTER this batch's
                    # sub so the in-order DVE stream never delays a sub
                    if b > 0:
                        nc.vector.tensor_copy(cop_sb[b - 1][:], ps_view(b - 1))
                    mms(b, 0, ND)
                else:
                    # split the last batch 6+2 tiles so the serial tail after
                    # the ACT chain ends is one small sub + 2 matmuls + copy
                    nc.scalar.activation(
                        logs[:, 1536:2688], lnin[:, 1536:2688], AF.Ln,
                        accum_out=am_sb[:, 3:4])
                    sub(b, 0, 6, nc.vector)
                    mms(b, 0, 3)
                    nc.scalar.activation(
                        logs[:, 2688:3072], lnin[:, 2688:3072], AF.Ln,
                        accum_out=am_sb[:, 4:5])
                    sub(b, 6, NT, nc.vector)
                    mms(b, 3, ND)
            # batches 2 and 3's copies on the ACT engine, which is idle once
            # its Ln chain ends; keeps the DVE tail free for batch-3 subs
            nc.scalar.copy(cop_sb[NB - 2][:], ps_view(NB - 2))
            nc.scalar.copy(cop_sb[NB - 1][:], ps_view(NB - 1))

            # ---- phase C: output DMAs
            nc.sync.dma_start(am_d, am_sb[:])
            for b in range(NB):
                nc.sync.dma_start(cop_d[:, b * 768:(b + 1) * 768],
                                  cop_sb[b][:])

    nc.compile()
    return nc


def _get_program():
    global _PROG
    if _PROG is None:
        _PROG = _build_program()
    return _PROG


def kernel(outputs, targets, attention_mask):
    global LAST
    out_np = np.asarray(outputs, dtype=np.float32).reshape(B, S, F)
    tgt_np = np.asarray(targets, dtype=np.float32).reshape(B, S, F)
    m_np = np.asarray(attention_mask)

    mf = m_np.astype(np.float32)[:, :, None]
    f8 = ml_dtypes.float8_e4m3fn
    # masked Ln inputs; binaries and masked copies are cheap host prep.
    # lnin = [xoo_b | xzo_b] per batch, in exact ACT consumption order.
    xoo_all = (out_np * mf + (1.0 - mf)).astype(f8)
    xzo_all = ((1.0 - out_np) * mf + (1.0 - mf)).astype(f8)
    lnin_all = np.concatenate(
        [xoo_all.reshape(B, 1, NT, 128, F),
         xzo_all.reshape(B, 1, NT, 128, F)], axis=1)  # [B, 2, NT, 128, F]
    # xt tiles zero-padded to 256 cols: [hi f0:128 | lo f128:192 | 64 zeros]
    xt_all = np.zeros((B, NT, 128, 256), dtype=f8)
    xt_all[:, :, :, 0:F] = tgt_np.astype(f8).reshape(B, NT, 128, F)
    # comb image: xo tiles in cols 0:192 of each 384 block, zeros in mD slots
    xoc_all = np.zeros((B, NT, 128, 384), dtype=f8)
    xoc_all[:, :, :, 0:F] = out_np.astype(f8).reshape(B, NT, 128, F)

    in_maps = []
    for c in range(NCORE):
        bs = slice(c * NB, (c + 1) * NB)
        in_maps.append({
            "lnin": np.ascontiguousarray(
                lnin_all[bs].transpose(3, 0, 1, 2, 4).reshape(128, NB * 3072)),
            "xt": np.ascontiguousarray(
                xt_all[bs].transpose(2, 0, 1, 3).reshape(128, NB * 2048)),
            "xoc": np.ascontiguousarray(
                xoc_all[bs].transpose(2, 0, 1, 3).reshape(128, NB * 3072)),
        })

    nc = _get_program()
    res = run_bass_kernel_spmd(nc, in_maps, list(range(NCORE)))
    LAST = res

    P = np.array(list(permutations(range(E))), dtype=np.int32)
    t_idx = np.arange(E)[None, :]
    ar = np.arange(E)
    p_arange = np.arange(128)
    diag = p_arange[:, None] % CI + np.arange(E)[None, :] * CI  # [128, 6]
    num = 0.0
    for c in range(NCORE):
        cop = res.results[c]["cop"].astype(np.float32)  # [128, NB*768]
        am = res.results[c]["am"]                       # [128, 5] f32
        for b in range(NB):
            pane = cop[:, b * 768:(b + 1) * 768]
            # pane cols: [cost-hi 0:192 | G-hi 192:384 | cost-lo | G-lo]
            # wanted entry per partition: col g*32 + p%32 of each block
            ch = pane[p_arange[:, None], diag]               # [128, 6]
            gh = pane[p_arange[:, None], 192 + diag]
            cl = pane[p_arange[:64, None], 384 + diag[:64]]  # [64, 6]
            gl = pane[p_arange[:64, None], 576 + diag[:64]]
            cost = -np.concatenate(
                [ch.reshape(4, CI, 6).sum(1, dtype=np.float32),
                 cl.reshape(2, CI, 6).sum(1, dtype=np.float32)], axis=0)
            G = np.concatenate(
                [gh.reshape(4, CI, 6).sum(1, dtype=np.float32),
                 gl.reshape(2, CI, 6).sum(1, dtype=np.float32)], axis=0)

            totals = cost[t_idx, P].sum(-1, dtype=np.float32)
            perm = P[int(np.argmin(totals))]
            num += -0.5 * float(G[ar, perm].sum(dtype=np.float64))
        num += 0.5 * -am.sum(dtype=np.float64)

    den = float(m_np.sum())
    return np.float32(num / den)
